# revision 19
# baseline (speedup 1.0000x reference)
"""HMQ-quantized MLP (fc1 -> exact GELU -> fc2) on 8 TRN2 NeuronCores.

Strategy: data-parallel over the 16384 token rows (2048 rows/core).
The int8 fake-quant values are integers in [-127, 127], exactly representable
in bf16, and all dot-product partial sums stay far below 2^24 -- so the
dequantized GEMMs are computed EXACTLY as bf16 integer matmuls on the PE
array with fp32 PSUM accumulation, then scaled by s_a*s_w.  Rounding uses
the +/-1.5*2^23 magic-constant trick (matches jnp.round round-half-even).

Latency work (baseline 690 us; PE floor at the throttled 13/16 clock is
~539 us, so everything else is preamble/transition/tail):
 - single-queue absmax pass (one DMA queue sustains ~300 GB/s; splitting
   across queues measured ~60 GB/s each), w-shards streamed in 1MB pieces
   through the x staging ring; scale-collective triggers at ~65us, under
   the ~60-95us runtime rendezvous barrier that gates the first collective
 - x read once: 4 chunks SBUF-resident, 4 re-read during the collective
   wait; quantization split across ACT+DVE (gpsimd compute is 5-10x slow)
 - fc1 starts with oc0+oc1 interleaved across all 8 PSUM banks so the PE
   tolerates the post-collective x-quantize ramp without stalling
 - gelu output staged to DRAM as fp16 (not f32): halves the 64 MB gT round
   trip; adds ~3e-3 rel err (gate is 2e-2), numerically validated
 - the fc2 quantization scale comes from the pre-GELU PSUM max (gelu is
   monotone where it matters, so max|g| == gelu(max h) exactly): the CC2
   trigger skips the gelu ACT chain and the scale matches reference f32
 - fc2 quantizes gq in [128,8,128] grains so the first matmul issues ~1.5us
   after the second collective lands
"""

import numpy as np

import concourse.bass as bass
import concourse.mybir as mybir
import concourse.tile as tile
from concourse import bacc, bass_isa
from concourse.bass_utils import run_bass_kernel_spmd

F32 = mybir.dt.float32
BF16 = mybir.dt.bfloat16
FP16 = mybir.dt.float16
ts = bass.ts

C_MAGIC = 1.5 * 2**23  # round-to-nearest-even for |v| < 2^22
QMAX = 127.0

NCORES = 8
B, T, D, H = 4, 4096, 1024, 4096
M = B * T            # 16384 total rows
S = M // NCORES      # 2048 rows per core

N_IC = D // 128      # 8  contraction chunks for fc1
N_OC = H // 128      # 32 output chunks for fc1 (hidden)
N_ST = S // 512      # 4  row tiles of 512
N_SC = S // 128      # 16 row chunks of 128
N_NC = H // 128      # 32 contraction chunks for fc2
N_JT = D // 512      # 2  output col tiles for fc2

# fc1 consumes x chunks in this order: resident chunks (4..7) first, then
# the re-read chunks (0..3) as they land.  Integer sums: order-exact.
IC_ORDER = [4, 5, 6, 7, 0, 1, 2, 3]

Copy = mybir.ActivationFunctionType.Copy
Gelu = mybir.ActivationFunctionType.Gelu
X_AX = mybir.AxisListType.X
XY_AX = mybir.AxisListType.XY
MAX = mybir.AluOpType.max
MULT = mybir.AluOpType.mult
SUB = mybir.AluOpType.subtract
ADD = mybir.AluOpType.add
RG = [list(range(NCORES))]


def build():
    nc = bacc.Bacc("TRN2", target_bir_lowering=False, debug=False,
                   num_devices=NCORES)

    xts = nc.dram_tensor("xts", [D, S], F32, kind="ExternalInput")
    w1t = nc.dram_tensor("w1t", [D, H], F32, kind="ExternalInput")
    w1s = nc.dram_tensor("w1s", [H // NCORES, D], F32, kind="ExternalInput")
    w2t = nc.dram_tensor("w2t", [H, D], F32, kind="ExternalInput")
    w2s = nc.dram_tensor("w2s", [D // NCORES, H], F32, kind="ExternalInput")
    b1m = nc.dram_tensor("b1m", [H // 128, 128], F32, kind="ExternalInput")
    b2m = nc.dram_tensor("b2m", [1, D], F32, kind="ExternalInput")
    out = nc.dram_tensor("out", [S, D], F32, kind="ExternalOutput")

    with tile.TileContext(nc) as tc:
        with (
            tc.tile_pool(name="misc", bufs=1) as misc,
            tc.tile_pool(name="xres", bufs=4) as xresp,
            tc.tile_pool(name="xtmp", bufs=2) as xtmpp,
            tc.tile_pool(name="xq", bufs=1) as xqp,
            tc.tile_pool(name="w1stage", bufs=3) as w1sp,
            tc.tile_pool(name="w1q", bufs=3) as w1qp,
            tc.tile_pool(name="w2stage", bufs=2) as w2sp,
            tc.tile_pool(name="w2q", bufs=1) as w2qp,
            tc.tile_pool(name="gout", bufs=4) as goutp,
            tc.tile_pool(name="qtmp", bufs=1) as qtmpp,
            tc.tile_pool(name="gq", bufs=2) as gqp,
            tc.tile_pool(name="outp", bufs=2) as outp,
            tc.tile_pool(name="psum", bufs=8, space="PSUM") as psump,
            tc.tile_pool(name="dram", bufs=1, space="DRAM") as dramp,
        ):
            # ---------------- persistent DRAM intermediates ----------------
            gT = dramp.tile([H, S], FP16, tag="gT")
            cc1_in = dramp.tile([1, 4], F32, tag="cc1i")
            cc1_out = dramp.tile([NCORES, 4], F32, tag="cc1o")
            cc2_in = dramp.tile([1, 1], F32, tag="cc2i")
            cc2_out = dramp.tile([NCORES, 1], F32, tag="cc2o")

            # ---------------- bias prep (scalar DMA queue) ----------------
            id32 = misc.tile([32, 32], F32, tag="id32")
            from concourse.masks import make_identity
            make_identity(nc, id32)
            b1sb = misc.tile([H // 128, 128], F32, tag="b1sb")
            nc.scalar.dma_start(out=b1sb, in_=b1m[:, :])
            b1ps = psump.tile([128, H // 128], F32, tag="mm")
            nc.tensor.transpose(b1ps, b1sb, id32)
            b1all = misc.tile([128, H // 128], F32, tag="b1all")
            nc.vector.tensor_copy(b1all, b1ps)

            b2row = misc.tile([1, D], F32, tag="b2row")
            nc.scalar.dma_start(out=b2row, in_=b2m[:, :])

            # ---------------- local abs-max pass (3 parallel queues) -------
            # part1 cols: 0..7 x chunks (in IC_ORDER position), 8 w1, 9 w2
            part1 = misc.tile([128, 24], F32, tag="part1")
            xst = {}
            # resident chunks 4..7 on the sync queue
            for i, ic in enumerate(IC_ORDER[:4]):
                xc = xresp.tile([128, S], F32, tag="xr", name=f"xc{ic}")
                nc.sync.dma_start(out=xc, in_=xts[ic * 128:(ic + 1) * 128, :])
                xst[ic] = xc
            # transient chunks 0..3 and the four 1MB w1/w2 absmax pieces
            # share the xtmp ring (8KB slots) on the same single sync queue
            # (one queue moves ~300 GB/s; splitting across queues measured
            # ~60 GB/s each).  Emission order c0,c1,w1,w1,c2,c3,w2,w2 keeps
            # every ring-slot eviction dep two transfers old => no stalls.
            def _xalloc(ic):
                xc = xtmpp.tile([128, S], F32, tag="xt", name=f"xc{ic}")
                nc.sync.dma_start(out=xc, in_=xts[ic * 128:(ic + 1) * 128, :])
                xst[ic] = xc

            wh = []

            def _w1alloc(i):
                wt = xtmpp.tile([128, 2, D], F32, tag="xt", name=f"w1p{i}")
                nc.sync.dma_start(
                    out=wt,
                    in_=w1s[i * 256:(i + 1) * 256, :].rearrange(
                        "(a p) d -> p a d", p=128))
                wh.append(wt)

            def _w2alloc(i):
                wt = xtmpp.tile([128, 2048], F32, tag="xt", name=f"w2p{i}")
                nc.sync.dma_start(out=wt,
                                  in_=w2s[:, i * 2048:(i + 1) * 2048])
                wh.append(wt)

            _xalloc(0)
            _xalloc(1)
            _w1alloc(0)
            _w1alloc(1)
            _xalloc(2)
            _xalloc(3)
            _w2alloc(0)
            _w2alloc(1)

            # x reduces on DVE, emitted in approximate landing order
            # (x chunks stream on two queues in parallel)
            xcol = {ic: i for i, ic in enumerate(IC_ORDER)}

            def _xred(ic):
                nc.vector.tensor_reduce(out=part1[:, xcol[ic]:xcol[ic] + 1],
                                        in_=xst[ic], axis=X_AX, op=MAX,
                                        apply_absolute_value=True)

            def _wred(i):
                ax = XY_AX if wh[i].ndim == 3 else X_AX
                nc.vector.tensor_reduce(out=part1[:, 8 + i:9 + i], in_=wh[i],
                                        axis=ax, op=MAX,
                                        apply_absolute_value=True)

            for ic in (4, 5, 6, 7, 0, 1):
                _xred(ic)
            _wred(0)
            _wred(1)
            _xred(2)
            _xred(3)
            _wred(2)
            _wred(3)

            # combine -> arow cols [Mx, Mw1, Mw2, pad]
            arow = misc.tile([128, 4], F32, tag="arow")
            nc.vector.tensor_reduce(out=arow[:, 0:1], in_=part1[:, 0:8],
                                    axis=X_AX, op=MAX)
            nc.vector.tensor_reduce(out=arow[:, 1:2], in_=part1[:, 8:10],
                                    axis=X_AX, op=MAX)
            nc.vector.tensor_reduce(out=arow[:, 2:3], in_=part1[:, 10:12],
                                    axis=X_AX, op=MAX)
            nc.vector.tensor_copy(arow[:, 3:4], arow[:, 2:3])
            armax = misc.tile([128, 4], F32, tag="armax")
            nc.gpsimd.partition_all_reduce(armax, arow, channels=128,
                                           reduce_op=bass_isa.ReduceOp.max)

            # ------------- AllGather #1: global Mx, Mw1, Mw2 ---------------
            nc.gpsimd.dma_start(out=cc1_in, in_=armax[0:1, :])
            nc.gpsimd.collective_compute(
                "AllGather", mybir.AluOpType.bypass, replica_groups=RG,
                ins=[cc1_in.opt()], outs=[cc1_out.opt()])

            # re-reads of chunks 0..1 + first w1 chunks; all run during the
            # collective wait (chunks 2..3 re-read later -- their ring slots
            # only free once xrr0/xrr1 are quantized)
            xrr = {}
            for ic in (0, 1):
                xr = xtmpp.tile([128, S], F32, tag="xt", name=f"xrr{ic}")
                nc.sync.dma_start(out=xr, in_=xts[ic * 128:(ic + 1) * 128, :])
                xrr[ic] = xr
            w1c0 = w1sp.tile([128, N_IC, 128], F32, tag="w1c", name="w1c0")
            nc.sync.dma_start(
                out=w1c0,
                in_=w1t[:, ts(0, 128)].rearrange("(ic p) o -> p ic o", p=128))
            w1c1 = w1sp.tile([128, N_IC, 128], F32, tag="w1c", name="w1c1")
            nc.sync.dma_start(
                out=w1c1,
                in_=w1t[:, ts(1, 128)].rearrange("(ic p) o -> p ic o", p=128))

            # post-collective: global maxes -> scales
            g1g = misc.tile([NCORES, 4], F32, tag="g1g")
            nc.gpsimd.dma_start(out=g1g, in_=cc1_out[:, :])
            g1m = misc.tile([NCORES, 4], F32, tag="g1m")
            nc.gpsimd.partition_all_reduce(g1m, g1g, channels=NCORES,
                                           reduce_op=bass_isa.ReduceOp.max)
            g1 = misc.tile([128, 4], F32, tag="g1")
            nc.gpsimd.partition_broadcast(g1, g1m)
            b2r = misc.tile([128, D], F32, tag="b2r")
            nc.gpsimd.partition_broadcast(b2r, b2row)

            # scl cols: 0 sx | 1 sw1 | 2 sw2 | 3 inv_sx | 4 inv_sw1
            #           5 inv_sw2 | 6 d1
            scl = misc.tile([128, 8], F32, tag="scl")
            nc.vector.tensor_scalar(out=scl[:, 0:3], in0=g1[:, 0:3],
                                    scalar1=1e-8, scalar2=1.0 / QMAX,
                                    op0=MAX, op1=MULT)
            nc.vector.reciprocal(scl[:, 3:6], scl[:, 0:3])
            nc.vector.tensor_mul(scl[:, 6:7], scl[:, 0:1], scl[:, 1:2])

            # ---------------- quantize x + first two w1 chunks -------------
            # split across ACT / DVE / GpSimd so the PE ramp isn't starved
            xqT = xqp.tile([128, N_IC, S], BF16, tag="xq")

            def q_act(dst, src, inv, n=1):
                """ACT magic-round in place (f32), DVE subtract -> bf16."""
                nel = src.shape[-1] * (src.shape[-2] if src.ndim == 3 else 1)
                sf = src.rearrange("p a b -> p (a b)") if src.ndim == 3 else src
                df = dst.rearrange("p a b -> p (a b)") if dst.ndim == 3 else dst
                step = nel // n
                for k in range(n):
                    sl = slice(k * step, (k + 1) * step)
                    nc.scalar.activation(sf[:, sl], sf[:, sl], Copy,
                                         bias=C_MAGIC, scale=inv)
                    nc.vector.tensor_scalar(out=df[:, sl], in0=sf[:, sl],
                                            scalar1=C_MAGIC, scalar2=None,
                                            op0=SUB)

            def q_vec(eng, dst, src, inv):
                """two-op path on DVE or GpSimd: mult+magic, then subtract."""
                eng.tensor_scalar(out=src, in0=src, scalar1=inv,
                                  scalar2=C_MAGIC, op0=MULT, op1=ADD)
                eng.tensor_scalar(out=dst, in0=src, scalar1=C_MAGIC,
                                  scalar2=None, op0=SUB)

            w1q0 = w1qp.tile([128, N_IC, 128], BF16, tag="w1q", name="w1q0")
            w1q1 = w1qp.tile([128, N_IC, 128], BF16, tag="w1q", name="w1q1")
            # ACT stream, interleaved by need-time (fc1 consumes ic 4..7
            # first, so the [4:8] w1q halves and chunk 4 quantize first)
            q_act(w1q0[:, 4:8], w1c0[:, 4:8], scl[:, 4:5])          # 0.5us
            q_act(xqT[:, 4, :], xst[4], scl[:, 3:4], n=4)           # quarters
            q_act(w1q1[:, 4:8], w1c1[:, 4:8], scl[:, 4:5])
            q_vec(nc.vector, xqT[:, 5, :], xst[5], scl[:, 3:4])     # DVE pair
            q_act(xqT[:, 6, :], xst[6], scl[:, 3:4])
            q_act(xqT[:, 7, :], xst[7], scl[:, 3:4])
            q_vec(nc.vector, xqT[:, 0, :], xrr[0], scl[:, 3:4])
            q_act(xqT[:, 1, :], xrr[1], scl[:, 3:4])
            q_act(w1q0[:, 0:4], w1c0[:, 0:4], scl[:, 4:5])
            q_act(w1q1[:, 0:4], w1c1[:, 0:4], scl[:, 4:5])
            # late re-reads: DMA issues live on the gpsimd queue because they
            # must WAIT for xrr0/xrr1's quantize (slot eviction) -- putting
            # them on sync/scalar would stall those queues' later work.
            xrr2t = xresp.tile([128, S], F32, tag="xr", name="xrr2")
            nc.gpsimd.dma_start(out=xrr2t, in_=xts[2 * 128:3 * 128, :])
            xrr3t = xresp.tile([128, S], F32, tag="xr", name="xrr3")
            nc.gpsimd.dma_start(out=xrr3t, in_=xts[3 * 128:4 * 128, :])
            q_act(xqT[:, 2, :], xrr2t, scl[:, 3:4])
            q_vec(nc.vector, xqT[:, 3, :], xrr3t, scl[:, 3:4])
            del xrr

            # ---------------- fc1 ------------------------------------------
            # h^T = w1q @ xq^T; gelu; stage g^T to DRAM as fp16.  w2 load +
            # quantize interleaved one chunk per oc (paces the 16 MB of w2
            # reads across fc1).
            w2qT = w2qp.tile([128, N_NC, D], BF16, tag="w2q")
            # h-max path: gelu is monotone increasing above its dip and the
            # global max h is >>1, so max|g| == gelu(max h) exactly.  Taking
            # the max from PSUM (signed, pre-gelu, pre-bias) keeps the CC2
            # trigger off the gelu ACT chain, and makes the fc2 scale match
            # the reference f32 value (not the fp16-stored one).
            gpartS = misc.tile([128, N_ST], F32, tag="gpartS")
            gpartH = misc.tile([128, N_OC], F32, tag="gpartH")
            gtmp = misc.tile([128, 1], F32, tag="gtmp")

            def gelu_store(oc, pts):
                for st in range(N_ST):
                    go = goutp.tile([128, 512], FP16, tag="gout",
                                    name=f"go{oc}_{st}")
                    nc.scalar.activation(go, pts[st], Gelu,
                                         bias=b1all[:, oc:oc + 1],
                                         scale=scl[:, 6:7])
                    nc.vector.tensor_reduce(out=gpartS[:, st:st + 1],
                                            in_=pts[st], axis=X_AX, op=MAX)
                    nc.sync.dma_start(out=gT[ts(oc, 128), ts(st, 512)], in_=go)
                nc.vector.tensor_reduce(out=gtmp, in_=gpartS, axis=X_AX,
                                        op=MAX)
                nc.vector.tensor_scalar(out=gpartH[:, oc:oc + 1], in0=gtmp,
                                        scalar1=scl[:, 6:7],
                                        scalar2=b1all[:, oc:oc + 1],
                                        op0=MULT, op1=ADD)

            def w2_prep(oc):
                w2c = w2sp.tile([128, D], F32, tag="w2c", name=f"w2c{oc}")
                nc.scalar.dma_start(out=w2c, in_=w2t[ts(oc, 128), :])
                q_act(w2qT[:, oc, :], w2c, scl[:, 5:6])

            # oc0 + oc1 interleaved over all 8 PSUM banks: halves the early
            # xq consumption rate so the quantize ramp keeps up
            pts0 = [psump.tile([128, 512], F32, tag="mm", name=f"pt0_{st}")
                    for st in range(N_ST)]
            pts1 = [psump.tile([128, 512], F32, tag="mm", name=f"pt1_{st}")
                    for st in range(N_ST)]
            for ic in IC_ORDER:
                for w1q_, pts in ((w1q0, pts0), (w1q1, pts1)):
                    for st in range(N_ST):
                        nc.tensor.matmul(pts[st], lhsT=w1q_[:, ic, :],
                                         rhs=xqT[:, ic, ts(st, 512)],
                                         start=(ic == IC_ORDER[0]),
                                         stop=(ic == IC_ORDER[-1]))
            gelu_store(0, pts0)
            gelu_store(1, pts1)
            w2_prep(0)
            w2_prep(1)

            for oc in range(2, N_OC):
                w1c = w1sp.tile([128, N_IC, 128], F32, tag="w1c",
                                name=f"w1c{oc}")
                nc.sync.dma_start(
                    out=w1c,
                    in_=w1t[:, ts(oc, 128)].rearrange("(ic p) o -> p ic o",
                                                      p=128))
                w1q = w1qp.tile([128, N_IC, 128], BF16, tag="w1q",
                                name=f"w1q{oc}")
                q_act(w1q, w1c, scl[:, 4:5])
                pts = [psump.tile([128, 512], F32, tag="mm",
                                  name=f"pt{oc}_{st}") for st in range(N_ST)]
                for ic in IC_ORDER:
                    for st in range(N_ST):
                        nc.tensor.matmul(pts[st], lhsT=w1q[:, ic, :],
                                         rhs=xqT[:, ic, ts(st, 512)],
                                         start=(ic == IC_ORDER[0]),
                                         stop=(ic == IC_ORDER[-1]))
                gelu_store(oc, pts)
                # the last two w2 preps move past the CC2 trigger emission:
                # their DVE subs would otherwise sit ahead of the garow
                # reduce in the DVE FIFO and delay the collective by ~4us
                if oc < N_OC - 2:
                    w2_prep(oc)

            # ---------------- AllGather #2: global Mg ----------------------
            garow = misc.tile([128, 1], F32, tag="garow")
            nc.vector.tensor_reduce(out=garow, in_=gpartH, axis=X_AX, op=MAX)
            gamax = misc.tile([128, 1], F32, tag="gamax")
            nc.gpsimd.partition_all_reduce(gamax, garow, channels=128,
                                           reduce_op=bass_isa.ReduceOp.max)
            nc.gpsimd.dma_start(out=cc2_in, in_=gamax[0:1, :])
            nc.gpsimd.collective_compute(
                "AllGather", mybir.AluOpType.bypass, replica_groups=RG,
                ins=[cc2_in.opt()], outs=[cc2_out.opt()])
            w2_prep(N_OC - 2)
            w2_prep(N_OC - 1)

            g2g = misc.tile([NCORES, 1], F32, tag="g2g")
            nc.gpsimd.dma_start(out=g2g, in_=cc2_out[:, :])
            g2m = misc.tile([NCORES, 1], F32, tag="g2m")
            nc.gpsimd.partition_all_reduce(g2m, g2g, channels=NCORES,
                                           reduce_op=bass_isa.ReduceOp.max)
            g2 = misc.tile([128, 1], F32, tag="g2")
            nc.gpsimd.partition_broadcast(g2, g2m)

            # scl2 cols: 0 sg | 1 inv_sg | 2 d2.  g2 col0 carries the
            # global max h; apply exact-erf gelu to get max|g|.
            mg = misc.tile([128, 1], F32, tag="mg")
            nc.scalar.activation(mg, g2, Gelu, bias=0.0, scale=1.0)
            scl2 = misc.tile([128, 4], F32, tag="scl2")
            nc.vector.tensor_scalar(out=scl2[:, 0:1], in0=mg,
                                    scalar1=1e-8, scalar2=1.0 / QMAX,
                                    op0=MAX, op1=MULT)
            nc.vector.reciprocal(scl2[:, 1:2], scl2[:, 0:1])
            nc.vector.tensor_mul(scl2[:, 2:3], scl2[:, 0:1], scl[:, 2:3])

            # ---------------- fc2: out = gq^T.T @ w2q^T --------------------
            # gts staging reuses the xres ring (tag xr, slot >= gts size).
            for sc in range(N_SC):
                gq_halves = []
                for half in range(2):
                    nch = N_NC // 2
                    gts = xresp.tile([128, nch, 128], FP16, tag="xr",
                                     name=f"gts{sc}_{half}")
                    nc.sync.dma_start(
                        out=gts,
                        in_=gT[half * nch * 128:(half + 1) * nch * 128,
                               ts(sc, 128)].rearrange("(a p) s -> p a s",
                                                      p=128))
                    gq = gqp.tile([128, nch, 128], BF16, tag="gq",
                                  name=f"gq{sc}_{half}")
                    ngrain = 4 if (sc == 0 and half == 0) else 2
                    for q2 in range(ngrain):
                        sl = slice(q2 * (nch // ngrain),
                                   (q2 + 1) * (nch // ngrain))
                        tmp = qtmpp.tile([128, nch // ngrain, 128], F32,
                                         tag="qt",
                                         name=f"qt{sc}_{half}_{q2}")
                        nc.scalar.activation(
                            tmp.rearrange("p a b -> p (a b)"),
                            gts[:, sl, :].rearrange("p a b -> p (a b)"),
                            Copy, bias=C_MAGIC, scale=scl2[:, 1:2])
                        nc.vector.tensor_scalar(
                            out=gq[:, sl, :].rearrange("p a b -> p (a b)"),
                            in0=tmp.rearrange("p a b -> p (a b)"),
                            scalar1=C_MAGIC, scalar2=None, op0=SUB)
                    gq_halves.append(gq)
                pos = [psump.tile([128, 512], F32, tag="mm",
                                  name=f"po{sc}_{jt}") for jt in range(N_JT)]
                for half in range(2):
                    for k in range(N_NC // 2):
                        ncg = half * (N_NC // 2) + k
                        for jt in range(N_JT):
                            nc.tensor.matmul(pos[jt],
                                             lhsT=gq_halves[half][:, k, :],
                                             rhs=w2qT[:, ncg, ts(jt, 512)],
                                             start=(ncg == 0),
                                             stop=(ncg == N_NC - 1))
                for jt in range(N_JT):
                    ot = outp.tile([128, 512], F32, tag="ot",
                                   name=f"ot{sc}_{jt}")
                    nc.scalar.activation(ot, pos[jt], Copy, bias=0.0,
                                         scale=scl2[:, 2:3])
                    nc.vector.tensor_add(ot, ot, b2r[:, ts(jt, 512)])
                    nc.sync.dma_start(out=out[ts(sc, 128), ts(jt, 512)],
                                      in_=ot)

    nc.compile()
    return nc


_NC_CACHE = None


def _get_nc():
    global _NC_CACHE
    if _NC_CACHE is None:
        _NC_CACHE = build()
    return _NC_CACHE


def make_in_maps(x, w1, b1, w2, b2):
    xf = np.ascontiguousarray(x.reshape(M, D).T)          # [D, M]
    w1t_h = np.ascontiguousarray(w1.T)                    # [D, H]
    w2t_h = np.ascontiguousarray(w2.T)                    # [H, D]
    b1m_h = np.ascontiguousarray(b1.reshape(H // 128, 128))
    b2m_h = np.ascontiguousarray(b2.reshape(1, D))
    in_maps = []
    for c in range(NCORES):
        in_maps.append({
            "xts": np.ascontiguousarray(xf[:, c * S:(c + 1) * S]),
            "w1t": w1t_h,
            "w1s": np.ascontiguousarray(w1[c * (H // NCORES):(c + 1) * (H // NCORES), :]),
            "w2t": w2t_h,
            "w2s": np.ascontiguousarray(w2[c * (D // NCORES):(c + 1) * (D // NCORES), :]),
            "b1m": b1m_h,
            "b2m": b2m_h,
        })
    return in_maps


def kernel(x, w1, b1, w2, b2, _trace=False):
    nc = _get_nc()
    in_maps = make_in_maps(np.asarray(x, dtype=np.float32),
                           np.asarray(w1, dtype=np.float32),
                           np.asarray(b1, dtype=np.float32),
                           np.asarray(w2, dtype=np.float32),
                           np.asarray(b2, dtype=np.float32))
    res = run_bass_kernel_spmd(nc, in_maps, core_ids=list(range(NCORES)),
                               trace=_trace)
    full = np.concatenate([res.results[c]["out"] for c in range(NCORES)], axis=0)
    out = full.reshape(B, T, D)
    if _trace:
        kernel.last_results = res
    return out
